# revision 1
# baseline (speedup 1.0000x reference)
"""BatchBlobLoss Trainium2 kernel (8-core SPMD).

Reference computation:
  p = softmax(predictions, axis=1)[:, 1:]          # foreground class probs
  per (b, c): segment-sum of p keyed by instance id t = targets[b, c]
  soft-dice per (b, c, instance), masked mean -> scalar.

Device strategy (per core; cores k = 0..7 get batch b = k//4 and
D-slices 16*(k%4) .. +16):
  The 33-bin segment sum is computed with one fused reduce-op per bin:
    x = t + p  (p in (0,1) strictly, so floor(x) = t)
    ACT (scalar engine):  G_m  = sum relu(x - m)       = B_m + sum_{i>m} N_{>=i}
    ACT (Sign):           S_m  = sum sign(x - m)       = 2*N_{>=m} - n
    DVE (is_ge + accum):  N_{>=m} = sum [x >= m]
  where B_m = sum_{t >= m} p. Host (float64) recovers
    P_m = B_m - B_{m+1}  (per-instance prob sums) and C_m = N_{>=m} - N_{>=m+1}
  and evaluates the tiny dice formula. Per-partition accumulator strips
  [128, n_cols] are DMA'd out and reduced on host.
"""
import numpy as np
from contextlib import ExitStack

import concourse.bass as bass
import concourse.tile as tile
from concourse import bacc, mybir
from concourse import bass_utils
from concourse.bass_interp import get_hw_module

# problem shape (hardcoded per contest rules)
B, C, D, H, W = 2, 3, 64, 256, 256
M = 32
EPS = 1e-5
N_CORES = 8
CORES_PER_BATCH = 4
D_SH = D // CORES_PER_BATCH      # 16 depth slices per core
P = 128
NVOX = D_SH * H * W              # 1,048,576 voxels per core per channel
F = NVOX // P                    # 8192
CHUNK = 4096
NCHUNK = F // CHUNK              # 2
NBINS = 33                       # ids 0..32

# engine split for the 65 binning passes per channel (full 8192-rows)
VAL_SPLIT = list(range(0, 6))     # value bins binned per-chunk (overlap prep)
VAL_ACT = list(range(6, NBINS))   # value bins via ACT Relu -> G_m (full row)
VAL_DVE = []                      # value bins via DVE (sub,max) -> G_m
CNT_SPLIT_DVE = [1, 2, 3]         # count bins per-chunk on raw t (DVE is_ge)
CNT_ACT = [30, 31, 32]            # count bins per-chunk on raw t (ACT Sign)
CNT_HALF = 4                      # chunk0 on DVE, chunk1 on ACT Sign
CNT_DVE = list(range(5, 30))      # count bins via DVE is_ge on x (full row)

COLS_PER_SET = 65                 # 33 value + 32 count columns
N_VSPLIT_COLS = 2 * len(VAL_SPLIT)   # second-chunk cols for split value bins
N_CSPLIT = CNT_SPLIT_DVE + CNT_ACT + [CNT_HALF]  # per-chunk count bins
N_CSPLIT_COLS = 2 * len(N_CSPLIT)    # second-chunk cols for split count bins
N_COLS = 2 * COLS_PER_SET + N_VSPLIT_COLS + N_CSPLIT_COLS

F32 = mybir.dt.float32
BF16 = mybir.dt.bfloat16
I32 = mybir.dt.int32


def _val_col(ch, m):
    return ch * COLS_PER_SET + m


def _cnt_col(ch, m):
    return ch * COLS_PER_SET + NBINS + (m - 1)


def _split_col(ch, i):
    # chunk-1 partial for VAL_SPLIT[i]; chunk-0 partial lives in _val_col
    return 2 * COLS_PER_SET + ch * len(VAL_SPLIT) + i


def _csplit_col(ch, i):
    # chunk-1 partial for N_CSPLIT[i]; chunk-0 partial lives in _cnt_col
    return (2 * COLS_PER_SET + N_VSPLIT_COLS + ch * len(N_CSPLIT) + i)


def build_nc(scopes=False):
    AluOp = mybir.AluOpType
    ACT = mybir.ActivationFunctionType

    import contextlib

    def sc(nc, name):
        return nc.named_scope(name) if scopes else contextlib.nullcontext()

    nc = bacc.Bacc("TRN2", target_bir_lowering=False, debug=False,
                   num_devices=N_CORES)
    pred = nc.dram_tensor("pred", [C, P, F], F32, kind="ExternalInput").ap()
    targ = nc.dram_tensor("targ", [2, P, F], I32, kind="ExternalInput").ap()
    out = nc.dram_tensor("out", [P, N_COLS], F32, kind="ExternalOutput").ap()
    out_a = nc.dram_tensor("out_a", [P, N_COLS], F32,
                           kind="ExternalOutput").ap()

    with tile.TileContext(nc) as tc:
        with ExitStack() as ctx:
            pool = ctx.enter_context(tc.tile_pool(name="main", bufs=1))

            # bias strip: column m holds -m (f32), for ACT bias
            bias_i = pool.tile([P, NBINS], I32, tag="bias_i")
            nc.gpsimd.iota(bias_i[:], [[1, NBINS]], channel_multiplier=0)
            bias_f = pool.tile([P, NBINS], F32, tag="bias_f")
            nc.vector.tensor_scalar(bias_f[:], bias_i[:], -1.0, None, AluOp.mult)
            # half-shifted bias for Sign on raw integer t: sign(t - m + 0.5)
            bias_h = pool.tile([P, NBINS], F32, tag="bias_h")
            nc.vector.tensor_scalar(bias_h[:], bias_f[:], 0.5, None, AluOp.add)

            strip = pool.tile([P, N_COLS], F32, tag="strip")
            strip_a = pool.tile([P, N_COLS], F32, tag="strip_a")
            nc.gpsimd.memset(strip[:], 0.0)
            nc.gpsimd.memset(strip_a[:], 0.0)

            ones = pool.tile([P, 1], F32, tag="ones")
            nc.gpsimd.memset(ones[:], 1.0)

            trash_a = pool.tile([P, F], BF16, tag="trash_a")
            trash_d = pool.tile([P, F], BF16, tag="trash_d")

            xp1 = pool.tile([P, F], F32, tag="xp1")
            xp2 = pool.tile([P, F], F32, tag="xp2")

            for chunk in range(NCHUNK):
                sl = bass.ts(chunk, CHUNK)
                # load logits + targets for this chunk
                x0 = pool.tile([P, CHUNK], F32, tag="x0", bufs=1)
                x1 = pool.tile([P, CHUNK], F32, tag="x1", bufs=1)
                x2 = pool.tile([P, CHUNK], F32, tag="x2", bufs=1)
                t1 = pool.tile([P, CHUNK], I32, tag="t1", bufs=1)
                t2 = pool.tile([P, CHUNK], I32, tag="t2", bufs=1)
                scr = pool.tile([P, CHUNK], F32, tag="scr", bufs=1)
                nc.sync.dma_start(x0[:], pred[0, :, sl])
                nc.sync.dma_start(x1[:], pred[1, :, sl])
                nc.sync.dma_start(x2[:], pred[2, :, sl])
                nc.sync.dma_start(t1[:], targ[0, :, sl])
                nc.sync.dma_start(t2[:], targ[1, :, sl])

                with sc(nc, f"prep_exp{chunk}"):
                    # in-place exp: x_c <- e_c
                    nc.scalar.activation(x0[:], x0[:], ACT.Exp)
                    nc.scalar.activation(x1[:], x1[:], ACT.Exp)
                    nc.scalar.activation(x2[:], x2[:], ACT.Exp)
                with sc(nc, f"prep_dve{chunk}"):
                    # s = e0 + e1 + e2 into scr
                    nc.vector.tensor_tensor(scr[:], x0[:], x1[:], AluOp.add)
                    nc.vector.tensor_tensor(scr[:], scr[:], x2[:], AluOp.add)
                    # r = 1/s into x0 (e0 dead)
                    nc.vector.reciprocal_approx_fast(x0[:], scr[:])
                    # p1, p2 in place
                    nc.vector.tensor_tensor(x1[:], x1[:], x0[:], AluOp.mult)
                    nc.vector.tensor_tensor(x2[:], x2[:], x0[:], AluOp.mult)
                    # packed x = t + p into the full-row tiles
                    nc.vector.scalar_tensor_tensor(
                        xp1[:, sl], t1[:], 0.0, x1[:], AluOp.add, AluOp.add)
                    nc.vector.scalar_tensor_tensor(
                        xp2[:, sl], t2[:], 0.0, x2[:], AluOp.add, AluOp.add)

                # count bins on the raw int32 targets -- these only need the
                # t DMA, so they fill the engine-idle windows before/during
                # softmax prep
                with sc(nc, f"cnt_t{chunk}"):
                    for ch, tc_ in ((0, t1), (1, t2)):
                        for i, m in enumerate(CNT_SPLIT_DVE):
                            col = (_cnt_col(ch, m) if chunk == 0
                                   else _csplit_col(ch, i))
                            nc.vector.scalar_tensor_tensor(
                                trash_d[:, sl], tc_[:], float(m),
                                ones[:].to_broadcast((P, CHUNK)),
                                AluOp.is_ge, AluOp.mult,
                                accum_out=strip[:, col:col + 1])
                        for j, m in enumerate(CNT_ACT):
                            i = len(CNT_SPLIT_DVE) + j
                            col = (_cnt_col(ch, m) if chunk == 0
                                   else _csplit_col(ch, i))
                            nc.scalar.activation(
                                trash_a[:, sl], tc_[:], ACT.Sign,
                                bias=bias_h[:, m:m + 1], scale=1.0,
                                accum_out=strip_a[:, col:col + 1])

                # split value bins: bin this chunk's halves now so ACT has
                # work while the other chunk is being prepped
                with sc(nc, f"bin_split{chunk}"):
                    for ch, xpc in ((0, xp1), (1, xp2)):
                        for i, m in enumerate(VAL_SPLIT):
                            col = (_val_col(ch, m) if chunk == 0
                                   else _split_col(ch, i))
                            nc.scalar.activation(
                                trash_a[:, sl], xpc[:, sl], ACT.Relu,
                                bias=bias_f[:, m:m + 1], scale=1.0,
                                accum_out=strip_a[:, col:col + 1])

            # binning over the full [P, F] packed rows
            # ACT: all Relu ops first (both channels), then all Sign ops --
            # minimizes activation-table switches.
            with sc(nc, "bin_act_v"):
                for ch, xc in ((0, xp1), (1, xp2)):
                    for m in VAL_ACT:
                        nc.scalar.activation(
                            trash_a[:], xc[:], ACT.Relu,
                            bias=bias_f[:, m:m + 1], scale=1.0,
                            accum_out=strip_a[:, _val_col(ch, m):
                                              _val_col(ch, m) + 1])
            with sc(nc, "bin_dve"):
                for ch, xc in ((0, xp1), (1, xp2)):
                    for m in VAL_DVE:
                        nc.vector.tensor_scalar(
                            trash_d[:], xc[:], float(m), 0.0,
                            AluOp.subtract, AluOp.max,
                            accum_out=strip[:, _val_col(ch, m):
                                            _val_col(ch, m) + 1])
                    for m in CNT_DVE:
                        nc.vector.tensor_scalar(
                            trash_d[:], xc[:], float(m), 0.0,
                            AluOp.is_ge, AluOp.add,
                            accum_out=strip[:, _cnt_col(ch, m):
                                            _cnt_col(ch, m) + 1])

            # CNT_HALF: chunk-0 half on DVE, chunk-1 half on ACT Sign
            ih = N_CSPLIT.index(CNT_HALF)
            for ch, xc in ((0, xp1), (1, xp2)):
                c0 = _cnt_col(ch, CNT_HALF)
                c1 = _csplit_col(ch, ih)
                nc.vector.tensor_scalar(
                    trash_d[:, 0:CHUNK], xc[:, 0:CHUNK], float(CNT_HALF), 0.0,
                    AluOp.is_ge, AluOp.add,
                    accum_out=strip[:, c0:c0 + 1])
                nc.scalar.activation(
                    trash_a[:, CHUNK:2 * CHUNK], xc[:, CHUNK:2 * CHUNK],
                    ACT.Sign, bias=bias_f[:, CNT_HALF:CNT_HALF + 1], scale=1.0,
                    accum_out=strip_a[:, c1:c1 + 1])

            nc.sync.dma_start(out[:], strip[:])
            nc.sync.dma_start(out_a[:], strip_a[:])

    nc.compile()
    nc.m = get_hw_module(nc.m)
    return nc


_NC_CACHE = None


def _get_nc():
    global _NC_CACHE
    if _NC_CACHE is None:
        _NC_CACHE = build_nc()
    return _NC_CACHE


def make_in_maps(predictions, targets):
    in_maps = []
    for k in range(N_CORES):
        b = k // CORES_PER_BATCH
        d0 = (k % CORES_PER_BATCH) * D_SH
        pr = np.ascontiguousarray(
            predictions[b, :, d0:d0 + D_SH]).reshape(C, P, F)
        tg = np.ascontiguousarray(
            targets[b, 1:, d0:d0 + D_SH]).reshape(2, P, F)
        in_maps.append({"pred": pr, "targ": tg})
    return in_maps


def decode(strips):
    """strips: list of N_CORES arrays [P, N_COLS] -> final scalar (f64)."""
    n_row_elems = float(P * F)
    n_chunk_elems = float(P * CHUNK)
    Bv = np.zeros((B, 2, NBINS))       # B_m, m = 0..32
    Ng = np.zeros((B, 2, NBINS + 1))   # N_{>=m}, m = 1..33 (33 stays 0)
    Graw = np.zeros((B, 2, NBINS))
    for k in range(N_CORES):
        b = k // CORES_PER_BATCH
        s = strips[k].astype(np.float64).sum(axis=0)   # [N_COLS]
        for ch in range(2):
            for m in range(NBINS):
                Graw[b, ch, m] += s[_val_col(ch, m)]
            for i, m in enumerate(VAL_SPLIT):
                Graw[b, ch, m] += s[_split_col(ch, i)]
            for m in CNT_DVE:
                Ng[b, ch, m - 1] += s[_cnt_col(ch, m)]
            for i, m in enumerate(N_CSPLIT):
                c0 = s[_cnt_col(ch, m)]
                c1 = s[_csplit_col(ch, i)]
                if m in CNT_ACT:
                    Ng[b, ch, m - 1] += (0.5 * (c0 + n_chunk_elems)
                                         + 0.5 * (c1 + n_chunk_elems))
                elif m == CNT_HALF:
                    Ng[b, ch, m - 1] += c0 + 0.5 * (c1 + n_chunk_elems)
                else:
                    Ng[b, ch, m - 1] += c0 + c1
    # G_m = B_m + sum_{i>m} N_{>=i}  ->  B_m = G_m - suffix
    for b in range(B):
        for ch in range(2):
            for m in range(NBINS):
                # sum_{i>m} N_{>=i}: Ng index i-1 over i = m+1..33
                suffix_m = Ng[b, ch, m:NBINS].sum()
                Bv[b, ch, m] = Graw[b, ch, m] - suffix_m
    # P_m = B_m - B_{m+1};  C_m = N_{>=m} - N_{>=m+1}
    Pm = np.concatenate([Bv[:, :, :-1] - Bv[:, :, 1:], Bv[:, :, -1:]], axis=2)
    Cm = Ng[:, :, :NBINS - 1] - Ng[:, :, 1:NBINS]    # m = 1..32

    s_bg = Pm[:, :, 0:1]
    s_i = Pm[:, :, 1:]
    n_i = Cm
    dice = 1.0 - (2.0 * s_i + EPS) / (s_bg + s_i + n_i + EPS)
    present = (n_i > 0.5).astype(np.float64)
    per_class = (dice * present).sum(axis=(0, 2)) / np.maximum(
        present.sum(axis=(0, 2)), 1.0)
    return per_class.mean()


def kernel(predictions, targets):
    predictions = np.asarray(predictions, dtype=np.float32)
    targets = np.asarray(targets, dtype=np.int32)
    nc = _get_nc()
    in_maps = make_in_maps(predictions, targets)
    res = bass_utils.run_bass_kernel_spmd(
        nc, in_maps, core_ids=list(range(N_CORES)))
    strips = [res.results[k]["out"] + res.results[k]["out_a"]
              for k in range(N_CORES)]
    return np.float32(decode(strips))



# revision 5
# speedup vs baseline: 1.8217x; 1.8217x over previous
"""BatchBlobLoss Trainium2 kernel v2 (8-core SPMD): ACT+PE reducers.

Reference computation:
  p = softmax(predictions, axis=1)[:, 1:]
  per (b, c): segment-sum of p keyed by instance id t = targets[b, c]
  soft-dice per (b, c, instance), masked mean -> scalar.

All 130 reductions per core are relu-shift row sums:
  G_m = sum relu(x - m)  on x = t + p (fp16),  T_m = sum relu(t - m).
Decode on host:  P_ge = G - T;  S = dP_ge;  N_ge = dT;  C = dN_ge.

HW reality (measured): DVE's accumulate op (TENSOR_SCALAR_CACHE_REDUCE)
always runs 1x (~8.7us/row), while plain tensor_scalar fp16 runs 4x
(~2.2us/row) and ACT relu+accum costs ~7.4us/row. So:
  - ACT lane: relu+bias+accum bins directly (a bins).
  - PE lane: DVE computes relu feed at 4x, the PE reduces it with a
    ones-vector matmul into one PSUM row per bin, 512-column strided
    partials, accumulated over 16 matmuls (q bins, ~3.6us each on PE).
Host sums PSUM partials (512 per bin) / strip columns (128 partitions).
"""
import numpy as np
from contextlib import ExitStack

import concourse.bass as bass
import concourse.tile as tile
from concourse import bacc, mybir
from concourse import bass_utils
from concourse.bass_interp import get_hw_module

# problem shape (hardcoded per contest rules)
B, C, D, H, W = 2, 3, 64, 256, 256
M = 32
EPS = 1e-5
N_CORES = 8
CORES_PER_BATCH = 4
D_SH = D // CORES_PER_BATCH      # 16 depth slices per core
P = 128
NVOX = D_SH * H * W              # 1,048,576 voxels per core per channel
F = NVOX // P                    # 8192
CHUNK = 2048
NCHUNK = F // CHUNK              # 4
NBINS = 33                       # ids 0..32

N_G = NBINS                      # G bins per channel (m = 0..32)
N_T = NBINS - 1                  # T bins per channel (m = 0..31)
COLS_PER_CH = N_G + N_T          # 65
N_COLS = 2 * COLS_PER_CH         # 130

PSUM_W = 512                     # one PSUM bank of f32 per partition
MM_N = F // PSUM_W               # 16 matmuls per PE bin

F32 = mybir.dt.float32
BF16 = mybir.dt.bfloat16
FP16 = mybir.dt.float16

# lane sizes (tuned on HW): ACT gets ACT_T T fillers + ACT_G G bins,
# DVE-solo gets DVE_G G bins (slow cache-reduce, uses DVE slack), the
# PE lane takes the rest. Every FOLD_EVERY-th PE bin folds its feed in
# half on DVE first (halves PE stream at the cost of one DVE TT).
ACT_T = 6
ACT_G = 31
DVE_G = 0
FOLD_EVERY = 3


def _g_col(ch, m):
    return ch * COLS_PER_CH + m


def _t_col(ch, m):
    return ch * COLS_PER_CH + N_G + m


def _lanes():
    t_bins = [("T", ch, m) for ch in (0, 1) for m in range(N_T)]
    g_bins = [("G", ch, m) for ch in (0, 1) for m in range(N_G)]
    act_t = [("T", 0, 31), ("T", 1, 31), ("T", 1, 30), ("T", 0, 30),
             ("T", 1, 29), ("T", 0, 29), ("T", 1, 28), ("T", 0, 28)][:ACT_T]
    act_head = act_t[:1]
    act_rest_t = act_t[1:]
    take = {int(round(i * len(g_bins) / max(ACT_G, 1)))
            for i in range(ACT_G)}
    act_g = [b for i, b in enumerate(g_bins) if i in take]
    rest_g = [b for i, b in enumerate(g_bins) if i not in take]
    dve_g = rest_g[:DVE_G]
    pe_g = rest_g[DVE_G:]
    pe_t = [b for b in t_bins if b not in act_t]
    pe = pe_t + pe_g               # T feeds first: data ready earliest
    act = act_rest_t + act_g
    return act_head, act, dve_g, pe


def build_nc(scopes=False):
    AluOp = mybir.AluOpType
    ACT = mybir.ActivationFunctionType

    import contextlib

    def sc(nc, name):
        return nc.named_scope(name) if scopes else contextlib.nullcontext()

    act_head, act_bins, dve_bins, pe_bins = _lanes()
    assert len(act_head) + len(act_bins) + len(dve_bins) + len(pe_bins) \
        == N_COLS
    assert len(pe_bins) <= 128

    nc = bacc.Bacc("TRN2", target_bir_lowering=False, debug=False,
                   num_devices=N_CORES)
    pred = nc.dram_tensor("pred", [C, P, F], BF16, kind="ExternalInput").ap()
    targ = nc.dram_tensor("targ", [2, P, F], FP16, kind="ExternalInput").ap()
    out_d = nc.dram_tensor("out_d", [P, N_COLS], F32,
                           kind="ExternalOutput").ap()
    out_a = nc.dram_tensor("out_a", [P, N_COLS], F32,
                           kind="ExternalOutput").ap()
    out_p = nc.dram_tensor("out_p", [P, PSUM_W], F32,
                           kind="ExternalOutput").ap()

    with tile.TileContext(nc) as tc:
        with ExitStack() as ctx:
            pool = ctx.enter_context(tc.tile_pool(name="main", bufs=1))
            ppool = ctx.enter_context(tc.psum_pool(name="psum", bufs=1))

            # bias strip: column m holds -m (f32), for ACT Relu bias
            bias_i = pool.tile([P, NBINS], mybir.dt.int32, tag="bias_i")
            nc.gpsimd.iota(bias_i[:], [[1, NBINS]], channel_multiplier=0)
            bias_f = pool.tile([P, NBINS], F32, tag="bias_f")
            nc.vector.tensor_scalar(bias_f[:], bias_i[:], -1.0, None,
                                    AluOp.mult)

            strip_d = pool.tile([P, N_COLS], F32, tag="strip_d")
            strip_a = pool.tile([P, N_COLS], F32, tag="strip_a")
            nc.gpsimd.memset(strip_d[:], 0.0)
            nc.gpsimd.memset(strip_a[:], 0.0)
            # sliding-mask stationary: zeros except a ones column at 128;
            # lhsT for PE bin r = zo[:, 128-r : 256-r] puts the feed's
            # column-sums into PSUM row r (all other rows += 0).
            zo = pool.tile([P, 2 * P], FP16, tag="zo")
            nc.gpsimd.memset(zo[:], 0.0)
            nc.gpsimd.memset(zo[:, P:P + 1], 1.0)

            # full-row persistent tiles (fp16, 16KB/partition each)
            tb1 = pool.tile([P, F], FP16, tag="tb1")
            tb2 = pool.tile([P, F], FP16, tag="tb2")
            e1 = pool.tile([P, F], FP16, tag="e1")    # exp ch1 -> p1 -> xpk1
            e2 = pool.tile([P, F], FP16, tag="e2")    # exp ch2 -> p2 -> xpk2
            s = pool.tile([P, F], FP16, tag="s")      # sum of exps -> 1/s
            trash_a = pool.tile([P, F], mybir.dt.float8e4, tag="trash_a")
            trash_d = None
            if dve_bins:
                trash_d = pool.tile([P, F], FP16, tag="trash_d",
                                    name="trash_d")

            pt = ppool.tile([P, PSUM_W], F32, tag="pt")

            xpk = (e1, e2)
            tbs = (tb1, tb2)

            def act_bin(kind, ch, m):
                src_t = xpk[ch] if kind == "G" else tbs[ch]
                col = _g_col(ch, m) if kind == "G" else _t_col(ch, m)
                nc.scalar.activation(
                    trash_a[:], src_t[:], ACT.Relu,
                    bias=bias_f[:, m:m + 1], scale=1.0,
                    accum_out=strip_a[:, col:col + 1])

            def dve_bin(kind, ch, m):
                src_t = xpk[ch] if kind == "G" else tbs[ch]
                col = _g_col(ch, m) if kind == "G" else _t_col(ch, m)
                nc.vector.tensor_scalar(
                    trash_d[:], src_t[:], float(m), 0.0,
                    AluOp.subtract, AluOp.max,
                    accum_out=strip_d[:, col:col + 1])

            pe_first = [True]
            n_pe = len(pe_bins)

            def pe_bin(row, kind, ch, m, fold=False):
                src_t = xpk[ch] if kind == "G" else tbs[ch]
                feed = pool.tile([P, F], FP16, tag="feed", bufs=5)
                nc.vector.tensor_scalar(
                    feed[:], src_t[:], float(m), 0.0,
                    AluOp.subtract, AluOp.max)
                half = F // 2
                if fold:
                    nc.vector.tensor_tensor(
                        feed[:, :half], feed[:, :half], feed[:, half:],
                        AluOp.add)
                n_mm = (MM_N // 2) if fold else MM_N
                for t in range(n_mm):
                    nc.tensor.matmul(
                        pt[:, :],
                        zo[:, P - row:2 * P - row],
                        feed[:, t * PSUM_W:(t + 1) * PSUM_W],
                        start=pe_first[0],
                        stop=(row == n_pe - 1 and t == n_mm - 1))
                    pe_first[0] = False

            # DMA order: tb1 first (early T-bin work), pred chunk 0 next
            # (unblocks ACT exps), then tb2 + remaining pred chunks.
            nc.sync.dma_start(tb1[:], targ[0])
            for b in act_head:
                act_bin(*b)

            # softmax prep, chunked for DMA/compute overlap
            for chunk in range(NCHUNK):
                sl = bass.ts(chunk, CHUNK)
                x0 = pool.tile([P, CHUNK], BF16, tag="x0", bufs=2)
                x1 = pool.tile([P, CHUNK], BF16, tag="x1", bufs=2)
                x2 = pool.tile([P, CHUNK], BF16, tag="x2", bufs=2)
                e0 = pool.tile([P, CHUNK], FP16, tag="e0", bufs=2)
                nc.sync.dma_start(x0[:], pred[0, :, sl])
                nc.sync.dma_start(x1[:], pred[1, :, sl])
                nc.sync.dma_start(x2[:], pred[2, :, sl])
                if chunk == 0:
                    nc.sync.dma_start(tb2[:], targ[1])
                with sc(nc, f"prep_exp{chunk}"), tc.high_priority():
                    nc.scalar.activation(e0[:], x0[:], ACT.Exp)
                    nc.scalar.activation(e1[:, sl], x1[:], ACT.Exp)
                    nc.scalar.activation(e2[:, sl], x2[:], ACT.Exp)
                with sc(nc, f"prep_sum{chunk}"), tc.high_priority():
                    nc.vector.tensor_tensor(s[:, sl], e0[:], e1[:, sl],
                                            AluOp.add)
                    nc.vector.tensor_tensor(s[:, sl], s[:, sl], e2[:, sl],
                                            AluOp.add)

            # r = 1/s = exp(-ln(s)), in place
            with sc(nc, "prep_recip"), tc.high_priority():
                nc.scalar.activation(s[:], s[:], ACT.Ln)
                nc.scalar.activation(s[:], s[:], ACT.Exp, scale=-1.0)
            # p_c = e_c * r ; xpk_c = p_c + tb_c   (both in place on e_c)
            with sc(nc, "prep_pack"), tc.high_priority():
                nc.vector.tensor_tensor(e1[:], e1[:], s[:], AluOp.mult)
                nc.vector.tensor_tensor(e1[:], e1[:], tb1[:], AluOp.add)
                nc.vector.tensor_tensor(e2[:], e2[:], s[:], AluOp.mult)
                nc.vector.tensor_tensor(e2[:], e2[:], tb2[:], AluOp.add)

            # binning lanes
            strip_p = pool.tile([P, PSUM_W], F32, tag="strip_p")
            with sc(nc, "bin_pe"):
                for row, b in enumerate(pe_bins):
                    fold = bool(FOLD_EVERY) and (row % FOLD_EVERY
                                                 == FOLD_EVERY - 1)
                    pe_bin(row, *b, fold=fold)
                    if row == 63:
                        nc.vector.tensor_copy(strip_p[0:64, :], pt[0:64, :])
            with sc(nc, "bin_dve"):
                for b in dve_bins:
                    dve_bin(*b)
            with sc(nc, "bin_act"):
                for b in act_bins:
                    act_bin(*b)

            # evict remaining PSUM partials and ship strips
            nc.vector.tensor_copy(strip_p[64:P, :], pt[64:P, :])
            nc.sync.dma_start(out_p[:], strip_p[:])
            if dve_bins:
                nc.sync.dma_start(out_d[:], strip_d[:])
            nc.sync.dma_start(out_a[:], strip_a[:])

    nc.compile()
    # Exp/Ln/Relu all live in act-func-set 6 (natural_log_exp_and_others);
    # the auto-inserted loads pick first-containing sets (0 for exp, 5 for
    # ln) which churns the table RAM. Point every load at set 6.
    for f in nc.m.functions:
        for blk in f.blocks:
            for inst in blk.instructions:
                if isinstance(inst, mybir.InstLoadActFuncSet):
                    inst.act_func_set_id = 6
    nc.m = get_hw_module(nc.m)
    return nc


_NC_CACHE = None


def _get_nc():
    global _NC_CACHE
    if _NC_CACHE is None:
        _NC_CACHE = build_nc()
    return _NC_CACHE


def make_in_maps(predictions, targets):
    bf16 = mybir.dt.np(BF16)
    in_maps = []
    for k in range(N_CORES):
        b = k // CORES_PER_BATCH
        d0 = (k % CORES_PER_BATCH) * D_SH
        pr = np.ascontiguousarray(
            predictions[b, :, d0:d0 + D_SH]).reshape(C, P, F).astype(bf16)
        tg = np.ascontiguousarray(
            targets[b, 1:, d0:d0 + D_SH]).reshape(2, P, F).astype(np.float16)
        in_maps.append({"pred": pr, "targ": tg})
    return in_maps


def decode(results):
    """results: list of N_CORES dicts with out_d/out_a [P, N_COLS] and
    out_p [P, PSUM_W] -> final scalar (f64)."""
    _, _, _, pe_bins = _lanes()
    G = np.zeros((B, 2, NBINS + 1))      # G[m], m = 0..32; G[33] = 0
    T = np.zeros((B, 2, NBINS + 1))      # T[m], m = 0..31; T[32:] = 0
    for k in range(N_CORES):
        b = k // CORES_PER_BATCH
        ssum = results[k]["out_a"].astype(np.float64)
        if "out_d" in results[k]:
            ssum = ssum + results[k]["out_d"].astype(np.float64)
        ssum = ssum.sum(axis=0)
        psum = results[k]["out_p"].astype(np.float64).sum(axis=1)   # [128]
        for ch in range(2):
            for m in range(N_G):
                G[b, ch, m] += ssum[_g_col(ch, m)]
            for m in range(N_T):
                T[b, ch, m] += ssum[_t_col(ch, m)]
        for row, (kind, ch, m) in enumerate(pe_bins):
            if kind == "G":
                G[b, ch, m] += psum[row]
            else:
                T[b, ch, m] += psum[row]

    P_ge = G - T
    S = P_ge[:, :, :NBINS] - P_ge[:, :, 1:NBINS + 1]
    N_ge = T[:, :, 0:NBINS] - T[:, :, 1:NBINS + 1]
    Cnt = N_ge[:, :, :NBINS - 1] - N_ge[:, :, 1:NBINS]

    s_bg = S[:, :, 0:1]
    s_i = S[:, :, 1:]
    n_i = Cnt
    dice = 1.0 - (2.0 * s_i + EPS) / (s_bg + s_i + n_i + EPS)
    present = (n_i > 0.5).astype(np.float64)
    per_class = (dice * present).sum(axis=(0, 2)) / np.maximum(
        present.sum(axis=(0, 2)), 1.0)
    return per_class.mean()


def kernel(predictions, targets):
    predictions = np.asarray(predictions, dtype=np.float32)
    targets = np.asarray(targets, dtype=np.int32)
    nc = _get_nc()
    in_maps = make_in_maps(predictions, targets)
    res = bass_utils.run_bass_kernel_spmd(
        nc, in_maps, core_ids=list(range(N_CORES)))
    return np.float32(decode(res.results))


# revision 6
# speedup vs baseline: 1.8249x; 1.0017x over previous
"""BatchBlobLoss Trainium2 kernel v2 (8-core SPMD): ACT+PE reducers.

Reference computation:
  p = softmax(predictions, axis=1)[:, 1:]
  per (b, c): segment-sum of p keyed by instance id t = targets[b, c]
  soft-dice per (b, c, instance), masked mean -> scalar.

All 130 reductions per core are relu-shift row sums:
  G_m = sum relu(x - m)  on x = t + p (fp16),  T_m = sum relu(t - m).
Decode on host:  P_ge = G - T;  S = dP_ge;  N_ge = dT;  C = dN_ge.

HW reality (measured): DVE's accumulate op (TENSOR_SCALAR_CACHE_REDUCE)
always runs 1x (~8.7us/row), while plain tensor_scalar fp16 runs 4x
(~2.2us/row) and ACT relu+accum costs ~7.4us/row. So:
  - ACT lane: relu+bias+accum bins directly (a bins).
  - PE lane: DVE computes relu feed at 4x, the PE reduces it with a
    ones-vector matmul into one PSUM row per bin, 512-column strided
    partials, accumulated over 16 matmuls (q bins, ~3.6us each on PE).
Host sums PSUM partials (512 per bin) / strip columns (128 partitions).
"""
import numpy as np
from contextlib import ExitStack

import concourse.bass as bass
import concourse.tile as tile
from concourse import bacc, mybir
from concourse import bass_utils
from concourse.bass_interp import get_hw_module

# problem shape (hardcoded per contest rules)
B, C, D, H, W = 2, 3, 64, 256, 256
M = 32
EPS = 1e-5
N_CORES = 8
CORES_PER_BATCH = 4
D_SH = D // CORES_PER_BATCH      # 16 depth slices per core
P = 128
NVOX = D_SH * H * W              # 1,048,576 voxels per core per channel
F = NVOX // P                    # 8192
CHUNK = 2048
NCHUNK = F // CHUNK              # 4
NBINS = 33                       # ids 0..32

N_G = NBINS                      # G bins per channel (m = 0..32)
N_T = NBINS - 1                  # T bins per channel (m = 0..31)
COLS_PER_CH = N_G + N_T          # 65
N_COLS = 2 * COLS_PER_CH         # 130

PSUM_W = 512                     # one PSUM bank of f32 per partition
MM_N = F // PSUM_W               # 16 matmuls per PE bin

F32 = mybir.dt.float32
BF16 = mybir.dt.bfloat16
FP16 = mybir.dt.float16

# lane sizes (tuned on HW): ACT gets ACT_T T fillers + ACT_G G bins,
# DVE-solo gets DVE_G G bins (slow cache-reduce, uses DVE slack), the
# PE lane takes the rest. Every FOLD_EVERY-th PE bin folds its feed in
# half on DVE first (halves PE stream at the cost of one DVE TT).
ACT_T = 6
ACT_G = 30
DVE_G = 0
FOLD_EVERY = 3


def _g_col(ch, m):
    return ch * COLS_PER_CH + m


def _t_col(ch, m):
    return ch * COLS_PER_CH + N_G + m


def _lanes():
    t_bins = [("T", ch, m) for ch in (0, 1) for m in range(N_T)]
    g_bins = [("G", ch, m) for ch in (0, 1) for m in range(N_G)]
    act_t = [("T", 0, 31), ("T", 1, 31), ("T", 1, 30), ("T", 0, 30),
             ("T", 1, 29), ("T", 0, 29), ("T", 1, 28), ("T", 0, 28)][:ACT_T]
    act_head = act_t[:1]
    act_rest_t = act_t[1:]
    take = {int(round(i * len(g_bins) / max(ACT_G, 1)))
            for i in range(ACT_G)}
    act_g = [b for i, b in enumerate(g_bins) if i in take]
    rest_g = [b for i, b in enumerate(g_bins) if i not in take]
    dve_g = rest_g[:DVE_G]
    pe_g = rest_g[DVE_G:]
    pe_t = [b for b in t_bins if b not in act_t]
    pe = pe_t + pe_g               # T feeds first: data ready earliest
    act = act_rest_t + act_g
    return act_head, act, dve_g, pe


def build_nc(scopes=False):
    AluOp = mybir.AluOpType
    ACT = mybir.ActivationFunctionType

    import contextlib

    def sc(nc, name):
        return nc.named_scope(name) if scopes else contextlib.nullcontext()

    act_head, act_bins, dve_bins, pe_bins = _lanes()
    assert len(act_head) + len(act_bins) + len(dve_bins) + len(pe_bins) \
        == N_COLS
    assert len(pe_bins) <= 128

    nc = bacc.Bacc("TRN2", target_bir_lowering=False, debug=False,
                   num_devices=N_CORES)
    pred = nc.dram_tensor("pred", [C, P, F], BF16, kind="ExternalInput").ap()
    targ = nc.dram_tensor("targ", [2, P, F], FP16, kind="ExternalInput").ap()
    out_d = nc.dram_tensor("out_d", [P, N_COLS], F32,
                           kind="ExternalOutput").ap()
    out_a = nc.dram_tensor("out_a", [P, N_COLS], F32,
                           kind="ExternalOutput").ap()
    out_p = nc.dram_tensor("out_p", [P, PSUM_W], F32,
                           kind="ExternalOutput").ap()

    with tile.TileContext(nc) as tc:
        with ExitStack() as ctx:
            pool = ctx.enter_context(tc.tile_pool(name="main", bufs=1))
            ppool = ctx.enter_context(tc.psum_pool(name="psum", bufs=1))

            # bias strip: column m holds -m (f32), for ACT Relu bias
            bias_i = pool.tile([P, NBINS], mybir.dt.int32, tag="bias_i")
            nc.gpsimd.iota(bias_i[:], [[1, NBINS]], channel_multiplier=0)
            bias_f = pool.tile([P, NBINS], F32, tag="bias_f")
            nc.vector.tensor_scalar(bias_f[:], bias_i[:], -1.0, None,
                                    AluOp.mult)

            strip_d = pool.tile([P, N_COLS], F32, tag="strip_d")
            strip_a = pool.tile([P, N_COLS], F32, tag="strip_a")
            nc.gpsimd.memset(strip_d[:], 0.0)
            nc.gpsimd.memset(strip_a[:], 0.0)
            # sliding-mask stationary: zeros except a ones column at 128;
            # lhsT for PE bin r = zo[:, 128-r : 256-r] puts the feed's
            # column-sums into PSUM row r (all other rows += 0).
            zo = pool.tile([P, 2 * P], FP16, tag="zo")
            nc.gpsimd.memset(zo[:], 0.0)
            nc.gpsimd.memset(zo[:, P:P + 1], 1.0)

            # full-row persistent tiles (fp16, 16KB/partition each)
            tb1 = pool.tile([P, F], FP16, tag="tb1")
            tb2 = pool.tile([P, F], FP16, tag="tb2")
            e1 = pool.tile([P, F], FP16, tag="e1")    # exp ch1 -> p1 -> xpk1
            e2 = pool.tile([P, F], FP16, tag="e2")    # exp ch2 -> p2 -> xpk2
            s = pool.tile([P, F], FP16, tag="s")      # sum of exps -> 1/s
            trash_a = pool.tile([P, F], mybir.dt.float8e4, tag="trash_a")
            trash_d = None
            if dve_bins:
                trash_d = pool.tile([P, F], FP16, tag="trash_d",
                                    name="trash_d")

            pt = ppool.tile([P, PSUM_W], F32, tag="pt")

            xpk = (e1, e2)
            tbs = (tb1, tb2)

            def act_bin(kind, ch, m):
                src_t = xpk[ch] if kind == "G" else tbs[ch]
                col = _g_col(ch, m) if kind == "G" else _t_col(ch, m)
                nc.scalar.activation(
                    trash_a[:], src_t[:], ACT.Relu,
                    bias=bias_f[:, m:m + 1], scale=1.0,
                    accum_out=strip_a[:, col:col + 1])

            def dve_bin(kind, ch, m):
                src_t = xpk[ch] if kind == "G" else tbs[ch]
                col = _g_col(ch, m) if kind == "G" else _t_col(ch, m)
                nc.vector.tensor_scalar(
                    trash_d[:], src_t[:], float(m), 0.0,
                    AluOp.subtract, AluOp.max,
                    accum_out=strip_d[:, col:col + 1])

            pe_first = [True]
            n_pe = len(pe_bins)

            def pe_bin(row, kind, ch, m, fold=False, split_feed=False):
                src_t = xpk[ch] if kind == "G" else tbs[ch]
                feed = pool.tile([P, F], FP16, tag="feed", bufs=5)
                half = F // 2
                if split_feed:
                    nc.vector.tensor_scalar(
                        feed[:, :half], src_t[:, :half], float(m), 0.0,
                        AluOp.subtract, AluOp.max)
                    nc.vector.tensor_scalar(
                        feed[:, half:], src_t[:, half:], float(m), 0.0,
                        AluOp.subtract, AluOp.max)
                else:
                    nc.vector.tensor_scalar(
                        feed[:], src_t[:], float(m), 0.0,
                        AluOp.subtract, AluOp.max)
                if fold:
                    nc.vector.tensor_tensor(
                        feed[:, :half], feed[:, :half], feed[:, half:],
                        AluOp.add)
                n_mm = (MM_N // 2) if fold else MM_N
                for t in range(n_mm):
                    nc.tensor.matmul(
                        pt[:, :],
                        zo[:, P - row:2 * P - row],
                        feed[:, t * PSUM_W:(t + 1) * PSUM_W],
                        start=pe_first[0],
                        stop=(row == n_pe - 1 and t == n_mm - 1))
                    pe_first[0] = False

            # DMA order: tb1 halves first (early T-bin work), pred chunk 0
            # next (unblocks ACT exps), then tb2 + remaining pred chunks.
            half = F // 2
            nc.sync.dma_start(tb1[:, :half], targ[0][:, :half])
            nc.sync.dma_start(tb1[:, half:], targ[0][:, half:])
            for b in act_head:
                act_bin(*b)

            # softmax prep, chunked for DMA/compute overlap
            for chunk in range(NCHUNK):
                sl = bass.ts(chunk, CHUNK)
                x0 = pool.tile([P, CHUNK], BF16, tag="x0", bufs=2)
                x1 = pool.tile([P, CHUNK], BF16, tag="x1", bufs=2)
                x2 = pool.tile([P, CHUNK], BF16, tag="x2", bufs=2)
                e0 = pool.tile([P, CHUNK], FP16, tag="e0", bufs=2)
                nc.sync.dma_start(x0[:], pred[0, :, sl])
                nc.sync.dma_start(x1[:], pred[1, :, sl])
                nc.sync.dma_start(x2[:], pred[2, :, sl])
                if chunk == 0:
                    nc.sync.dma_start(tb2[:], targ[1])
                with sc(nc, f"prep_exp{chunk}"), tc.high_priority():
                    nc.scalar.activation(e0[:], x0[:], ACT.Exp)
                    nc.scalar.activation(e1[:, sl], x1[:], ACT.Exp)
                    nc.scalar.activation(e2[:, sl], x2[:], ACT.Exp)
                with sc(nc, f"prep_sum{chunk}"), tc.high_priority():
                    nc.vector.tensor_tensor(s[:, sl], e0[:], e1[:, sl],
                                            AluOp.add)
                    nc.vector.tensor_tensor(s[:, sl], s[:, sl], e2[:, sl],
                                            AluOp.add)

            # r = 1/s = exp(-ln(s)), in place
            with sc(nc, "prep_recip"), tc.high_priority():
                nc.scalar.activation(s[:], s[:], ACT.Ln)
                nc.scalar.activation(s[:], s[:], ACT.Exp, scale=-1.0)
            # p_c = e_c * r ; xpk_c = p_c + tb_c   (both in place on e_c)
            with sc(nc, "prep_pack"), tc.high_priority():
                nc.vector.tensor_tensor(e1[:], e1[:], s[:], AluOp.mult)
                nc.vector.tensor_tensor(e1[:], e1[:], tb1[:], AluOp.add)
                nc.vector.tensor_tensor(e2[:], e2[:], s[:], AluOp.mult)
                nc.vector.tensor_tensor(e2[:], e2[:], tb2[:], AluOp.add)

            # binning lanes
            strip_p = pool.tile([P, PSUM_W], F32, tag="strip_p")
            with sc(nc, "bin_pe"):
                for row, b in enumerate(pe_bins):
                    fold = bool(FOLD_EVERY) and (row % FOLD_EVERY
                                                 == FOLD_EVERY - 1)
                    pe_bin(row, *b, fold=fold, split_feed=(row < 2))
                    if row == 63:
                        nc.vector.tensor_copy(strip_p[0:64, :], pt[0:64, :])
            with sc(nc, "bin_dve"):
                for b in dve_bins:
                    dve_bin(*b)
            with sc(nc, "bin_act"):
                for b in act_bins:
                    act_bin(*b)

            # evict remaining PSUM partials and ship strips
            nc.vector.tensor_copy(strip_p[64:P, :], pt[64:P, :])
            nc.sync.dma_start(out_p[:], strip_p[:])
            if dve_bins:
                nc.sync.dma_start(out_d[:], strip_d[:])
            nc.sync.dma_start(out_a[:], strip_a[:])

    nc.compile()
    # Exp/Ln/Relu all live in act-func-set 6 (natural_log_exp_and_others);
    # the auto-inserted loads pick first-containing sets (0 for exp, 5 for
    # ln) which churns the table RAM. Point every load at set 6.
    for f in nc.m.functions:
        for blk in f.blocks:
            for inst in blk.instructions:
                if isinstance(inst, mybir.InstLoadActFuncSet):
                    inst.act_func_set_id = 6
    nc.m = get_hw_module(nc.m)
    return nc


_NC_CACHE = None


def _get_nc():
    global _NC_CACHE
    if _NC_CACHE is None:
        _NC_CACHE = build_nc()
    return _NC_CACHE


def make_in_maps(predictions, targets):
    bf16 = mybir.dt.np(BF16)
    in_maps = []
    for k in range(N_CORES):
        b = k // CORES_PER_BATCH
        d0 = (k % CORES_PER_BATCH) * D_SH
        pr = np.ascontiguousarray(
            predictions[b, :, d0:d0 + D_SH]).reshape(C, P, F).astype(bf16)
        tg = np.ascontiguousarray(
            targets[b, 1:, d0:d0 + D_SH]).reshape(2, P, F).astype(np.float16)
        in_maps.append({"pred": pr, "targ": tg})
    return in_maps


def decode(results):
    """results: list of N_CORES dicts with out_d/out_a [P, N_COLS] and
    out_p [P, PSUM_W] -> final scalar (f64)."""
    _, _, _, pe_bins = _lanes()
    G = np.zeros((B, 2, NBINS + 1))      # G[m], m = 0..32; G[33] = 0
    T = np.zeros((B, 2, NBINS + 1))      # T[m], m = 0..31; T[32:] = 0
    for k in range(N_CORES):
        b = k // CORES_PER_BATCH
        ssum = results[k]["out_a"].astype(np.float64)
        if "out_d" in results[k]:
            ssum = ssum + results[k]["out_d"].astype(np.float64)
        ssum = ssum.sum(axis=0)
        psum = results[k]["out_p"].astype(np.float64).sum(axis=1)   # [128]
        for ch in range(2):
            for m in range(N_G):
                G[b, ch, m] += ssum[_g_col(ch, m)]
            for m in range(N_T):
                T[b, ch, m] += ssum[_t_col(ch, m)]
        for row, (kind, ch, m) in enumerate(pe_bins):
            if kind == "G":
                G[b, ch, m] += psum[row]
            else:
                T[b, ch, m] += psum[row]

    P_ge = G - T
    S = P_ge[:, :, :NBINS] - P_ge[:, :, 1:NBINS + 1]
    N_ge = T[:, :, 0:NBINS] - T[:, :, 1:NBINS + 1]
    Cnt = N_ge[:, :, :NBINS - 1] - N_ge[:, :, 1:NBINS]

    s_bg = S[:, :, 0:1]
    s_i = S[:, :, 1:]
    n_i = Cnt
    dice = 1.0 - (2.0 * s_i + EPS) / (s_bg + s_i + n_i + EPS)
    present = (n_i > 0.5).astype(np.float64)
    per_class = (dice * present).sum(axis=(0, 2)) / np.maximum(
        present.sum(axis=(0, 2)), 1.0)
    return per_class.mean()


def kernel(predictions, targets):
    predictions = np.asarray(predictions, dtype=np.float32)
    targets = np.asarray(targets, dtype=np.int32)
    nc = _get_nc()
    in_maps = make_in_maps(predictions, targets)
    res = bass_utils.run_bass_kernel_spmd(
        nc, in_maps, core_ids=list(range(N_CORES)))
    return np.float32(decode(res.results))


# revision 7
# speedup vs baseline: 1.8371x; 1.0067x over previous
"""BatchBlobLoss Trainium2 kernel v2 (8-core SPMD): ACT+PE reducers.

Reference computation:
  p = softmax(predictions, axis=1)[:, 1:]
  per (b, c): segment-sum of p keyed by instance id t = targets[b, c]
  soft-dice per (b, c, instance), masked mean -> scalar.

All 130 reductions per core are relu-shift row sums:
  G_m = sum relu(x - m)  on x = t + p (fp16),  T_m = sum relu(t - m).
Decode on host:  P_ge = G - T;  S = dP_ge;  N_ge = dT;  C = dN_ge.

HW reality (measured): DVE's accumulate op (TENSOR_SCALAR_CACHE_REDUCE)
always runs 1x (~8.7us/row), while plain tensor_scalar fp16 runs 4x
(~2.2us/row) and ACT relu+accum costs ~7.4us/row. So:
  - ACT lane: relu+bias+accum bins directly (a bins).
  - PE lane: DVE computes relu feed at 4x, the PE reduces it with a
    ones-vector matmul into one PSUM row per bin, 512-column strided
    partials, accumulated over 16 matmuls (q bins, ~3.6us each on PE).
Host sums PSUM partials (512 per bin) / strip columns (128 partitions).
"""
import numpy as np
from contextlib import ExitStack

import concourse.bass as bass
import concourse.tile as tile
from concourse import bacc, mybir
from concourse import bass_utils
from concourse.bass_interp import get_hw_module

# problem shape (hardcoded per contest rules)
B, C, D, H, W = 2, 3, 64, 256, 256
M = 32
EPS = 1e-5
N_CORES = 8
CORES_PER_BATCH = 4
D_SH = D // CORES_PER_BATCH      # 16 depth slices per core
P = 128
NVOX = D_SH * H * W              # 1,048,576 voxels per core per channel
F = NVOX // P                    # 8192
CHUNK = 2048
NCHUNK = F // CHUNK              # 4
NBINS = 33                       # ids 0..32

N_G = NBINS                      # G bins per channel (m = 0..32)
N_T = NBINS - 1                  # T bins per channel (m = 0..31)
COLS_PER_CH = N_G + N_T          # 65
N_COLS = 2 * COLS_PER_CH         # 130

PSUM_W = 512                     # one PSUM bank of f32 per partition
MM_N = F // PSUM_W               # 16 matmuls per PE bin

F32 = mybir.dt.float32
BF16 = mybir.dt.bfloat16
FP16 = mybir.dt.float16

# lane sizes (tuned on HW): ACT gets ACT_T T fillers + ACT_G G bins,
# DVE-solo gets DVE_G G bins (slow cache-reduce, uses DVE slack), the
# PE lane takes the rest. Every FOLD_EVERY-th PE bin folds its feed in
# half on DVE first (halves PE stream at the cost of one DVE TT).
ACT_T = 6
ACT_G = 30
DVE_G = 0
FOLD_EVERY = 3


def _g_col(ch, m):
    return ch * COLS_PER_CH + m


def _t_col(ch, m):
    return ch * COLS_PER_CH + N_G + m


def _lanes():
    t_bins = [("T", ch, m) for ch in (0, 1) for m in range(N_T)]
    g_bins = [("G", ch, m) for ch in (0, 1) for m in range(N_G)]
    act_t = [("T", 0, 31), ("T", 1, 31), ("T", 1, 30), ("T", 0, 30),
             ("T", 1, 29), ("T", 0, 29), ("T", 1, 28), ("T", 0, 28)][:ACT_T]
    act_head = act_t[:1]
    act_rest_t = act_t[1:]
    take = {int(round(i * len(g_bins) / max(ACT_G, 1)))
            for i in range(ACT_G)}
    act_g = [b for i, b in enumerate(g_bins) if i in take]
    rest_g = [b for i, b in enumerate(g_bins) if i not in take]
    dve_g = rest_g[:DVE_G]
    pe_g = rest_g[DVE_G:]
    pe_t = [b for b in t_bins if b not in act_t]
    pe = pe_t + pe_g               # T feeds first: data ready earliest
    act = act_rest_t + act_g
    return act_head, act, dve_g, pe


def build_nc(scopes=False):
    AluOp = mybir.AluOpType
    ACT = mybir.ActivationFunctionType

    import contextlib

    def sc(nc, name):
        return nc.named_scope(name) if scopes else contextlib.nullcontext()

    act_head, act_bins, dve_bins, pe_bins = _lanes()
    assert len(act_head) + len(act_bins) + len(dve_bins) + len(pe_bins) \
        == N_COLS
    assert len(pe_bins) <= 128

    nc = bacc.Bacc("TRN2", target_bir_lowering=False, debug=False,
                   num_devices=N_CORES)
    pred = nc.dram_tensor("pred", [C, P, F], BF16, kind="ExternalInput").ap()
    targ = nc.dram_tensor("targ", [2, P, F], FP16, kind="ExternalInput").ap()
    out_d = nc.dram_tensor("out_d", [P, N_COLS], F32,
                           kind="ExternalOutput").ap()
    out_a = nc.dram_tensor("out_a", [P, N_COLS], F32,
                           kind="ExternalOutput").ap()
    out_p = nc.dram_tensor("out_p", [P, PSUM_W], F32,
                           kind="ExternalOutput").ap()

    with tile.TileContext(nc) as tc:
        with ExitStack() as ctx:
            pool = ctx.enter_context(tc.tile_pool(name="main", bufs=1))
            ppool = ctx.enter_context(tc.psum_pool(name="psum", bufs=1))

            # bias strip: column m holds -m (f32), for ACT Relu bias
            bias_i = pool.tile([P, NBINS], mybir.dt.int32, tag="bias_i")
            nc.gpsimd.iota(bias_i[:], [[1, NBINS]], channel_multiplier=0)
            bias_f = pool.tile([P, NBINS], F32, tag="bias_f")
            nc.vector.tensor_scalar(bias_f[:], bias_i[:], -1.0, None,
                                    AluOp.mult)

            strip_d = pool.tile([P, N_COLS], F32, tag="strip_d")
            strip_a = pool.tile([P, N_COLS], F32, tag="strip_a")
            nc.gpsimd.memset(strip_d[:], 0.0)
            nc.gpsimd.memset(strip_a[:], 0.0)
            # sliding-mask stationary: zeros except a ones column at 128;
            # lhsT for PE bin r = zo[:, 128-r : 256-r] puts the feed's
            # column-sums into PSUM row r (all other rows += 0).
            zo = pool.tile([P, 2 * P], FP16, tag="zo")
            nc.gpsimd.memset(zo[:], 0.0)
            nc.gpsimd.memset(zo[:, P:P + 1], 1.0)

            # full-row persistent tiles (fp16, 16KB/partition each)
            tb1 = pool.tile([P, F], FP16, tag="tb1")
            tb2 = pool.tile([P, F], FP16, tag="tb2")
            e1 = pool.tile([P, F], FP16, tag="e1")    # exp ch1 -> p1 -> xpk1
            e2 = pool.tile([P, F], FP16, tag="e2")    # exp ch2 -> p2 -> xpk2
            s = pool.tile([P, F], FP16, tag="s")      # sum of exps -> 1/s
            trash_a = pool.tile([P, F], mybir.dt.float8e4, tag="trash_a")
            trash_d = None
            if dve_bins:
                trash_d = pool.tile([P, F], FP16, tag="trash_d",
                                    name="trash_d")

            pt = ppool.tile([P, PSUM_W], F32, tag="pt")

            xpk = (e1, e2)
            tbs = (tb1, tb2)

            def act_bin(kind, ch, m):
                src_t = xpk[ch] if kind == "G" else tbs[ch]
                col = _g_col(ch, m) if kind == "G" else _t_col(ch, m)
                nc.scalar.activation(
                    trash_a[:], src_t[:], ACT.Relu,
                    bias=bias_f[:, m:m + 1], scale=1.0,
                    accum_out=strip_a[:, col:col + 1])

            def dve_bin(kind, ch, m):
                src_t = xpk[ch] if kind == "G" else tbs[ch]
                col = _g_col(ch, m) if kind == "G" else _t_col(ch, m)
                nc.vector.tensor_scalar(
                    trash_d[:], src_t[:], float(m), 0.0,
                    AluOp.subtract, AluOp.max,
                    accum_out=strip_d[:, col:col + 1])

            pe_first = [True]
            n_pe = len(pe_bins)

            def pe_bin(row, kind, ch, m, fold=False, split_feed=False):
                src_t = xpk[ch] if kind == "G" else tbs[ch]
                if m == 0 and not fold:
                    # relu(x - 0) == x (both sources are non-negative):
                    # stream the source tile straight into the PE
                    for t in range(MM_N):
                        nc.tensor.matmul(
                            pt[:, :],
                            zo[:, P - row:2 * P - row],
                            src_t[:, t * PSUM_W:(t + 1) * PSUM_W],
                            start=pe_first[0], stop=False)
                        pe_first[0] = False
                    return
                feed = pool.tile([P, F], FP16, tag="feed", bufs=5)
                half = F // 2
                if split_feed:
                    nc.vector.tensor_scalar(
                        feed[:, :half], src_t[:, :half], float(m), 0.0,
                        AluOp.subtract, AluOp.max)
                    nc.vector.tensor_scalar(
                        feed[:, half:], src_t[:, half:], float(m), 0.0,
                        AluOp.subtract, AluOp.max)
                else:
                    nc.vector.tensor_scalar(
                        feed[:], src_t[:], float(m), 0.0,
                        AluOp.subtract, AluOp.max)
                if fold:
                    nc.vector.tensor_tensor(
                        feed[:, :half], feed[:, :half], feed[:, half:],
                        AluOp.add)
                n_mm = (MM_N // 2) if fold else MM_N
                for t in range(n_mm):
                    nc.tensor.matmul(
                        pt[:, :],
                        zo[:, P - row:2 * P - row],
                        feed[:, t * PSUM_W:(t + 1) * PSUM_W],
                        start=pe_first[0],
                        stop=(row == n_pe - 1 and t == n_mm - 1))
                    pe_first[0] = False

            # DMA order: tb1 halves first (early T-bin work), pred chunk 0
            # next (unblocks ACT exps), then tb2 + remaining pred chunks.
            half = F // 2
            nc.sync.dma_start(tb1[:, :half], targ[0][:, :half])
            nc.sync.dma_start(tb1[:, half:], targ[0][:, half:])
            for b in act_head:
                act_bin(*b)

            # softmax prep, chunked for DMA/compute overlap
            for chunk in range(NCHUNK):
                sl = bass.ts(chunk, CHUNK)
                x0 = pool.tile([P, CHUNK], BF16, tag="x0", bufs=2)
                x1 = pool.tile([P, CHUNK], BF16, tag="x1", bufs=2)
                x2 = pool.tile([P, CHUNK], BF16, tag="x2", bufs=2)
                e0 = pool.tile([P, CHUNK], FP16, tag="e0", bufs=2)
                nc.sync.dma_start(x0[:], pred[0, :, sl])
                nc.sync.dma_start(x1[:], pred[1, :, sl])
                nc.sync.dma_start(x2[:], pred[2, :, sl])
                if chunk == 0:
                    nc.sync.dma_start(tb2[:], targ[1])
                with sc(nc, f"prep_exp{chunk}"), tc.high_priority():
                    nc.scalar.activation(e0[:], x0[:], ACT.Exp)
                    nc.scalar.activation(e1[:, sl], x1[:], ACT.Exp)
                    nc.scalar.activation(e2[:, sl], x2[:], ACT.Exp)
                with sc(nc, f"prep_sum{chunk}"), tc.high_priority():
                    nc.vector.tensor_tensor(s[:, sl], e0[:], e1[:, sl],
                                            AluOp.add)
                    nc.vector.tensor_tensor(s[:, sl], s[:, sl], e2[:, sl],
                                            AluOp.add)

            # r = 1/s = exp(-ln(s)), in place
            with sc(nc, "prep_recip"), tc.high_priority():
                nc.scalar.activation(s[:], s[:], ACT.Ln)
                nc.scalar.activation(s[:], s[:], ACT.Exp, scale=-1.0)
            # p_c = e_c * r ; xpk_c = p_c + tb_c   (both in place on e_c)
            with sc(nc, "prep_pack"):
                with tc.high_priority():
                    nc.vector.tensor_tensor(e1[:], e1[:], s[:], AluOp.mult)
                    nc.vector.tensor_tensor(e1[:], e1[:], tb1[:], AluOp.add)
                nc.vector.tensor_tensor(e2[:], e2[:], s[:], AluOp.mult)
                nc.vector.tensor_tensor(e2[:], e2[:], tb2[:], AluOp.add)

            # binning lanes
            strip_p = pool.tile([P, PSUM_W], F32, tag="strip_p")
            with sc(nc, "bin_pe"):
                for row, b in enumerate(pe_bins):
                    fold = (bool(FOLD_EVERY)
                            and (row % FOLD_EVERY == FOLD_EVERY - 1)
                            and b[2] != 0)
                    pe_bin(row, *b, fold=fold, split_feed=(row < 2))
                    if row == 63:
                        nc.vector.tensor_copy(strip_p[0:64, :], pt[0:64, :])
            with sc(nc, "bin_dve"):
                for b in dve_bins:
                    dve_bin(*b)
            with sc(nc, "bin_act"):
                for b in act_bins:
                    act_bin(*b)

            # evict remaining PSUM partials and ship strips
            nc.vector.tensor_copy(strip_p[64:P, :], pt[64:P, :])
            nc.sync.dma_start(out_p[:], strip_p[:])
            if dve_bins:
                nc.sync.dma_start(out_d[:], strip_d[:])
            nc.sync.dma_start(out_a[:], strip_a[:])

    nc.compile()
    # Exp/Ln/Relu all live in act-func-set 6 (natural_log_exp_and_others);
    # the auto-inserted loads pick first-containing sets (0 for exp, 5 for
    # ln) which churns the table RAM. Point every load at set 6.
    for f in nc.m.functions:
        for blk in f.blocks:
            for inst in blk.instructions:
                if isinstance(inst, mybir.InstLoadActFuncSet):
                    inst.act_func_set_id = 6
    nc.m = get_hw_module(nc.m)
    return nc


_NC_CACHE = None


def _get_nc():
    global _NC_CACHE
    if _NC_CACHE is None:
        _NC_CACHE = build_nc()
    return _NC_CACHE


def make_in_maps(predictions, targets):
    bf16 = mybir.dt.np(BF16)
    in_maps = []
    for k in range(N_CORES):
        b = k // CORES_PER_BATCH
        d0 = (k % CORES_PER_BATCH) * D_SH
        pr = np.ascontiguousarray(
            predictions[b, :, d0:d0 + D_SH]).reshape(C, P, F).astype(bf16)
        tg = np.ascontiguousarray(
            targets[b, 1:, d0:d0 + D_SH]).reshape(2, P, F).astype(np.float16)
        in_maps.append({"pred": pr, "targ": tg})
    return in_maps


def decode(results):
    """results: list of N_CORES dicts with out_d/out_a [P, N_COLS] and
    out_p [P, PSUM_W] -> final scalar (f64)."""
    _, _, _, pe_bins = _lanes()
    G = np.zeros((B, 2, NBINS + 1))      # G[m], m = 0..32; G[33] = 0
    T = np.zeros((B, 2, NBINS + 1))      # T[m], m = 0..31; T[32:] = 0
    for k in range(N_CORES):
        b = k // CORES_PER_BATCH
        ssum = results[k]["out_a"].astype(np.float64)
        if "out_d" in results[k]:
            ssum = ssum + results[k]["out_d"].astype(np.float64)
        ssum = ssum.sum(axis=0)
        psum = results[k]["out_p"].astype(np.float64).sum(axis=1)   # [128]
        for ch in range(2):
            for m in range(N_G):
                G[b, ch, m] += ssum[_g_col(ch, m)]
            for m in range(N_T):
                T[b, ch, m] += ssum[_t_col(ch, m)]
        for row, (kind, ch, m) in enumerate(pe_bins):
            if kind == "G":
                G[b, ch, m] += psum[row]
            else:
                T[b, ch, m] += psum[row]

    P_ge = G - T
    S = P_ge[:, :, :NBINS] - P_ge[:, :, 1:NBINS + 1]
    N_ge = T[:, :, 0:NBINS] - T[:, :, 1:NBINS + 1]
    Cnt = N_ge[:, :, :NBINS - 1] - N_ge[:, :, 1:NBINS]

    s_bg = S[:, :, 0:1]
    s_i = S[:, :, 1:]
    n_i = Cnt
    dice = 1.0 - (2.0 * s_i + EPS) / (s_bg + s_i + n_i + EPS)
    present = (n_i > 0.5).astype(np.float64)
    per_class = (dice * present).sum(axis=(0, 2)) / np.maximum(
        present.sum(axis=(0, 2)), 1.0)
    return per_class.mean()


def kernel(predictions, targets):
    predictions = np.asarray(predictions, dtype=np.float32)
    targets = np.asarray(targets, dtype=np.int32)
    nc = _get_nc()
    in_maps = make_in_maps(predictions, targets)
    res = bass_utils.run_bass_kernel_spmd(
        nc, in_maps, core_ids=list(range(N_CORES)))
    return np.float32(decode(res.results))
